# revision 16
# baseline (speedup 1.0000x reference)
"""HGT (heterogeneous graph transformer) forward pass on 8 Trainium2 cores.

Strategy: pure data-parallel over batch (B=32 -> 4 per core), graph topology
replicated. The per-relation edge softmax + message aggregation is computed
DENSELY as masked attention over the 1000-node graph:

    ST_h[s, d]  = sum_f k_h[s, f] * q2_h[d, f]        (PE, heads packed in K)
    ST_h       += lnC[s, d]      (log edge-count mask; -100 for non-edges)
    E_h         = exp(ST_h)                            (ACT)
    agg_h[f, d] = sum_s v2_h[s, f] * E_h[s, d]         (PE; ones-col -> den)
    agg_h      /= max(den, 1e-12)

All activations are kept feature-major (features on partitions, nodes on the
free dim). Everything is fp32.

kernel(**inputs) takes the full unsharded inputs and returns (B, N, OUT).
"""

import math

import numpy as np

import concourse.bass as bass
import concourse.mybir as mybir
import concourse.tile as tile
from concourse.bass_utils import run_bass_kernel_spmd
from concourse.masks import make_identity

F32 = mybir.dt.float32
BF16 = mybir.dt.bfloat16
AF = mybir.ActivationFunctionType
ALU = mybir.AluOpType

# dims (fixed for this problem)
B, N, C, T = 32, 1000, 3, 12
R, HID, H, NB = 3, 128, 4, 2
DK = HID // H
E = 16000
OUT = 12
IN_DIM = C * T  # 36

NCORES = 8
BL = B // NCORES  # batch per core
NP = 1024        # padded node count
ST = NP // 128   # 8 src tiles
CH = NP // 512   # 2 dst chunks of 512
MASK_NEG = -100.0

# how many head-pairs get their mask via PE identity-matmul (rest via DVE add)
PE_MASK_HP = 2


class TC(tile.TileContext):
    """TileContext whose final drain splits sem waits one-per-instruction.

    walrus in this container rejects TPB_CTRL instructions carrying several
    sync waits ("Too many sync wait commands"); spread the global-clock waits
    across SP nops ahead of the drain.
    """

    def _drain_and_barrier(self, tick_clock, wait_clock):
        from concourse.vector_clock import ScopedClock

        nop0 = self.nc.sync.nop(nofuse=True)
        wait_clock.add_sem_waits(
            nop0.ins, ScopedClock({None: tick_clock.global_clock})
        )
        si = nop0.ins.sync_info
        if si is not None and len(si.on_wait) > 1:
            waits = list(si.on_wait)
            si.on_wait = waits[:1]
            for w in waits[1:]:
                n = self.nc.sync.nop(nofuse=True)
                n.ins.sync_info = mybir.SyncInfo(on_wait=[w], on_update=[])
        self.nc.sync.drain()
        self.nc.all_engine_barrier()
        assert self.sems is not None
        popped = self.nc._tile_sem_poison_stack.pop()
        assert popped is self._sem_poison
        self.nc.clear_and_free_semaphores(list(self.sems.allocated().values()))
        self.nc.all_engine_barrier()


# Max sync waits walrus's CoreV3 codegen will encode, by instruction type.
# Matmult lowers to LDWEIGHTS which takes a single wait; control-type
# instructions (drain/nop) likewise. Types observed compiling fine with 3
# keep 3; everything else conservatively 1.
_WAIT_CAPS = {}
_WAIT_CAP_DEFAULT = 1


def _split_sync_waits(nc):
    """walrus's CoreV3 codegen only encodes a few sync waits per
    instruction; move excess waits onto same-engine nops just ahead."""
    nid = [0]
    for f in nc.m.functions:
        for bb in f.blocks:
            out = []
            for inst in bb.instructions:
                tname = type(inst).__name__
                if tname == "InstEventSemaphore":
                    out.append(inst)
                    continue
                maxw = _WAIT_CAPS.get(tname, _WAIT_CAP_DEFAULT)
                si = inst.sync_info
                if si is not None and len(si.on_wait) > maxw:
                    waits = list(si.on_wait)
                    keep = waits[-maxw:]
                    for w in waits[:-maxw]:
                        nid[0] += 1
                        nop = mybir.InstNoOp(
                            name=f"WSPLIT-{nid[0]}", engine=inst.engine,
                            ins=[], outs=[], bass_nofuse=True,
                            sync_info=mybir.SyncInfo(on_wait=[w], on_update=[]),
                        )
                        nc.register_instruction(nop)
                        out.append(nop)
                    si.on_wait = keep
                out.append(inst)
            bb.instructions.clear()
            for i in out:
                bb.add_instruction(i)


def build_program(embed_gelu=True, stage=8):
    nc = bass.Bass()

    # ---- DRAM I/O -------------------------------------------------------
    xT_d = nc.declare_dram_parameter("xT", [BL, IN_DIM, NP], F32, isOutput=False)
    lnC_d = nc.declare_dram_parameter("lnC", [R, ST, 128, NP], BF16, isOutput=False)
    adaptWT_d = nc.declare_dram_parameter("adaptWT", [IN_DIM, HID], F32, isOutput=False)
    adaptb_d = nc.declare_dram_parameter("adaptb", [HID, 1], F32, isOutput=False)
    # per-layer weights, stacked on a leading layer axis
    WkT_d = nc.declare_dram_parameter("WkT", [NB, HID, HID], F32, isOutput=False)
    WqT_d = nc.declare_dram_parameter("WqT", [NB, HID, HID], F32, isOutput=False)
    WvT_d = nc.declare_dram_parameter("WvT", [NB, HID, HID], F32, isOutput=False)
    WaT_d = nc.declare_dram_parameter("WaT", [NB, HID, HID], F32, isOutput=False)
    bk_d = nc.declare_dram_parameter("bk", [NB, HID, 1], F32, isOutput=False)
    bq_d = nc.declare_dram_parameter("bq", [NB, HID, 1], F32, isOutput=False)
    bv_d = nc.declare_dram_parameter("bv", [NB, HID, 1], F32, isOutput=False)
    ba_d = nc.declare_dram_parameter("ba", [NB, HID, 1], F32, isOutput=False)
    BDatt_d = nc.declare_dram_parameter("BDatt", [NB, R, HID, HID], F32, isOutput=False)
    BDmsg_d = nc.declare_dram_parameter("BDmsg", [NB, R, HID, HID], F32, isOutput=False)
    gamma_d = nc.declare_dram_parameter("gamma", [NB, HID, 1], F32, isOutput=False)
    beta_d = nc.declare_dram_parameter("beta", [NB, HID, 1], F32, isOutput=False)
    a1m_d = nc.declare_dram_parameter("a1m", [NB, HID, 1], F32, isOutput=False)
    predWT_d = nc.declare_dram_parameter("predWT", [HID, HID], F32, isOutput=False)
    predb_d = nc.declare_dram_parameter("predb", [HID, 1], F32, isOutput=False)
    headWT_d = nc.declare_dram_parameter("headWT", [HID, OUT], F32, isOutput=False)
    headb_d = nc.declare_dram_parameter("headb", [OUT, 1], F32, isOutput=False)
    ieye_d = nc.declare_dram_parameter("ieye", [OUT, OUT], F32, isOutput=False)
    out_d = nc.declare_dram_parameter("out", [BL, NP, OUT], F32, isOutput=True)

    with TC(nc) as tc:
        with (
            tc.tile_pool(name="const", bufs=1) as constp,
            tc.tile_pool(name="act", bufs=1) as actp,
            tc.tile_pool(name="work", bufs=1) as workp,
            tc.tile_pool(name="escore", bufs=4) as ep,
            tc.tile_pool(name="v2np", bufs=16) as v2p,
            tc.tile_pool(name="small", bufs=4) as smallp,
            tc.tile_pool(name="ps", bufs=1, space="PSUM") as psp,
            tc.tile_pool(name="psagg", bufs=1, space="PSUM") as paggp,
            tc.tile_pool(name="psmisc", bufs=2, space="PSUM") as pmiscp,
        ):
            # ---- persistent constants ----------------------------------
            ident = constp.tile([128, 128], F32, tag="ident")
            make_identity(nc, ident[:])
            ones_m = constp.tile([128, 128], F32, tag="ones_m")
            nc.vector.memset(ones_m[:], 1.0 / HID)
            ones_row = constp.tile([1, 128], F32, tag="ones_row")
            nc.vector.memset(ones_row[:], 1.0)
            identBF = constp.tile([128, 128], BF16, tag="identBF")
            nc.vector.tensor_copy(out=identBF[:], in_=ident[:])

            lnC_sb = []
            for r in range(R):
                t = constp.tile([128, ST * NP], BF16, tag=f"lnC{r}")
                # DRAM [st, p, d] -> SBUF [p, st*NP + d]
                nc.sync.dma_start(
                    out=t[:].rearrange("p (s d) -> p s d", s=ST),
                    in_=lnC_d[r].rearrange("s p d -> p s d"),
                )
                lnC_sb.append(t)

            def load_w(dram_ap, shape, tag):
                t = constp.tile(list(shape), F32, tag=tag)
                nc.sync.dma_start(out=t[:], in_=dram_ap)
                return t

            adaptWT = load_w(adaptWT_d[:], (IN_DIM, HID), "adaptWT")
            adaptb = load_w(adaptb_d[:], (HID, 1), "adaptb")
            predWT = load_w(predWT_d[:], (HID, HID), "predWT")
            predb = load_w(predb_d[:], (HID, 1), "predb")
            headWT = load_w(headWT_d[:], (HID, OUT), "headWT")
            headb = load_w(headb_d[:], (OUT, 1), "headb")
            identOUT = load_w(ieye_d[:], (OUT, OUT), "identOUT")
            Wl = {}
            for l in range(NB):
                for nm, d in (
                    ("WkT", WkT_d), ("WqT", WqT_d), ("WvT", WvT_d), ("WaT", WaT_d),
                ):
                    Wl[nm, l] = load_w(d[l], (HID, HID), f"{nm}{l}")
                for nm, d in (
                    ("bk", bk_d), ("bq", bq_d), ("bv", bv_d), ("ba", ba_d),
                    ("gamma", gamma_d), ("beta", beta_d), ("a1m", a1m_d),
                ):
                    Wl[nm, l] = load_w(d[l], (HID, 1), f"{nm}{l}")
                for r in range(R):
                    Wl["BDatt", l, r] = load_w(BDatt_d[l, r], (HID, HID), f"BDatt{l}{r}")
                    Wl["BDmsg", l, r] = load_w(BDmsg_d[l, r], (HID, HID), f"BDmsg{l}{r}")

            # persistent scatter buffers for q2 (zeros outside head blocks)
            q2sc = []
            for c in range(CH):
                t = constp.tile([128, H * 512], F32, tag=f"q2sc{c}")
                nc.vector.memset(t[:], 0.0)
                q2sc.append(t)

            # ---- helpers ------------------------------------------------
            def linear(dst, srcT, WT, bias, func=AF.Identity):
                """dst (128, NP) = func(WT.T @ srcT + bias); all feature-major."""
                for c in range(CH):
                    sl = slice(c * 512, (c + 1) * 512)
                    ps = pmiscp.tile([128, 512], F32, tag="misc")
                    nc.tensor.matmul(
                        out=ps[:], lhsT=WT[:], rhs=srcT[:, sl], start=True, stop=True
                    )
                    nc.scalar.activation(dst[:, sl], ps[:], func, bias=bias[:])

            # ---- per-batch network -------------------------------------
            for b in range(BL):
                xT = actp.tile([IN_DIM, NP], F32, tag="xT")
                nc.sync.dma_start(out=xT[:], in_=xT_d[b])
                hT = actp.tile([HID, NP], F32, tag="hT")
                linear(hT, xT, adaptWT, adaptb, func=AF.Gelu if embed_gelu else AF.Identity)

                for l in range(NB):
                    if stage < 2:
                        break
                    kT = actp.tile([HID, NP], F32, tag="kT")
                    qT = actp.tile([HID, NP], F32, tag="qT")
                    vT = actp.tile([HID, NP], F32, tag="vT")
                    linear(kT, hT, Wl["WkT", l], Wl["bk", l])
                    linear(qT, hT, Wl["WqT", l], Wl["bq", l])
                    linear(vT, hT, Wl["WvT", l], Wl["bv", l])

                    agg_sb = [actp.tile([HID, NP], F32, tag=f"agg{r}", name=f"aggsb{r}") for r in range(R)]
                    if stage < 3:
                        continue

                    for r in range(R):
                        # q2T = BDatt @ qT (mu/sqrt(dk) folded in)
                        q2T = workp.tile([HID, NP], F32, tag="q2T")
                        for c in range(CH):
                            sl = slice(c * 512, (c + 1) * 512)
                            ps = pmiscp.tile([128, 512], F32, tag="misc")
                            nc.tensor.matmul(
                                out=ps[:], lhsT=Wl["BDatt", l, r][:], rhs=qT[:, sl],
                                start=True, stop=True,
                            )
                            nc.vector.tensor_copy(out=q2T[:, sl], in_=ps[:])

                        # scatter q2T into block-column layout for packed-K scores
                        for c in range(CH):
                            for h in range(H):
                                nc.vector.tensor_copy(
                                    out=q2sc[c][h * 32:(h + 1) * 32, h * 512:(h + 1) * 512],
                                    in_=q2T[h * 32:(h + 1) * 32, c * 512:(c + 1) * 512],
                                )

                        # v2 node-major (w_msg folded), with ones col per head
                        v2n = []
                        for st in range(ST):
                            ps = pmiscp.tile([128, 128], F32, tag="misc")
                            nc.tensor.matmul(
                                out=ps[:], lhsT=vT[:, st * 128:(st + 1) * 128],
                                rhs=Wl["BDmsg", l, r][:], start=True, stop=True,
                            )
                            t = v2p.tile([128, H * 33], F32, tag="v2n")
                            nc.vector.tensor_copy(
                                out=t[:].rearrange("p (h f) -> p h f", h=H)[:, :, 0:32],
                                in_=ps[:].rearrange("p (h f) -> p h f", h=H),
                            )
                            nc.vector.memset(
                                t[:].rearrange("p (h f) -> p h f", h=H)[:, :, 32:33], 1.0
                            )
                            v2n.append(t)

                        if stage < 4:
                            continue
                        for c in range(CH):
                            if stage < 5 and c > 0:
                                continue
                            aggps = [
                                paggp.tile([33, 512], F32, tag=f"aggps{h}", name=f"aggps{h}")
                                for h in range(H)
                            ]
                            for st in range(ST):
                                kslice = kT[:, st * 128:(st + 1) * 128]
                                etiles = []
                                for hp in range(2):
                                    ps = psp.tile([128, 1024], F32, tag="sc")
                                    for hh in range(2):
                                        h = hp * 2 + hh
                                        nc.tensor.matmul(
                                            out=ps[:, hh * 512:(hh + 1) * 512],
                                            lhsT=kslice,
                                            rhs=q2sc[c][:, h * 512:(h + 1) * 512],
                                            start=True, stop=(hp >= PE_MASK_HP),
                                        )
                                    lnslice = lnC_sb[r][:, st * NP + c * 512: st * NP + (c + 1) * 512]
                                    if hp < PE_MASK_HP:
                                        for hh in range(2):
                                            nc.tensor.matmul(
                                                out=ps[:, hh * 512:(hh + 1) * 512],
                                                lhsT=identBF[:], rhs=lnslice,
                                                start=False, stop=True,
                                            )
                                    else:
                                        lnrep = bass.AP(
                                            lnslice.tensor, lnslice.offset,
                                            [lnslice.ap[0], [0, 2], lnslice.ap[1]],
                                        )
                                        nc.vector.tensor_tensor(
                                            out=ps[:], in0=ps[:], in1=lnrep, op=ALU.add,
                                        )
                                    et = ep.tile([128, 1024], F32, tag="E")
                                    nc.scalar.activation(et[:], ps[:], AF.Exp)
                                    etiles.append(et)
                                for h in range(H if stage >= 5 else 0):
                                    nc.tensor.matmul(
                                        out=aggps[h][:],
                                        lhsT=v2n[st][:, h * 33:(h + 1) * 33],
                                        rhs=etiles[h // 2][:, (h % 2) * 512:(h % 2 + 1) * 512],
                                        start=(st == 0), stop=(st == ST - 1),
                                    )
                            # normalize: agg_sb[r][h-block, c] = agg / max(den,eps)
                            for h in range(H if stage >= 6 else 0):
                                den = smallp.tile([1, 512], F32, tag="den")
                                nc.vector.tensor_scalar_max(
                                    out=den[:], in0=aggps[h][32:33, :], scalar1=1e-12
                                )
                                rden = smallp.tile([1, 512], F32, tag="rden")
                                nc.vector.reciprocal(rden[:], den[:])
                                rdps = pmiscp.tile([32, 512], F32, tag="misc")
                                nc.tensor.matmul(
                                    out=rdps[:], lhsT=ones_row[:, 0:32], rhs=rden[:],
                                    start=True, stop=True,
                                )
                                rdb = smallp.tile([32, 512], F32, tag="rdb")
                                nc.vector.tensor_copy(out=rdb[:], in_=rdps[:])
                                nc.vector.tensor_tensor(
                                    out=agg_sb[r][h * 32:(h + 1) * 32, c * 512:(c + 1) * 512],
                                    in0=aggps[h][0:32, :], in1=rdb[:],
                                    op=ALU.mult,
                                )

                    if stage < 7:
                        continue
                    # trans = sum_r WaT.T @ agg_r  (alpha/3 folded); + ba
                    transb = workp.tile([HID, NP], F32, tag="transb")
                    for c in range(CH):
                        sl = slice(c * 512, (c + 1) * 512)
                        ps = pmiscp.tile([128, 512], F32, tag="misc")
                        for r in range(R):
                            nc.tensor.matmul(
                                out=ps[:], lhsT=Wl["WaT", l][:], rhs=agg_sb[r][:, sl],
                                start=(r == 0), stop=(r == R - 1),
                            )
                        nc.scalar.activation(transb[:, sl], ps[:], AF.Identity, bias=Wl["ba", l][:])

                    # out_pre = transb + (1-alpha) * h
                    outpre = workp.tile([HID, NP], F32, tag="outpre")
                    nc.vector.tensor_scalar(
                        out=outpre[:], in0=hT[:], scalar1=Wl["a1m", l][:],
                        scalar2=None, op0=ALU.mult,
                    )
                    nc.vector.tensor_tensor(
                        out=outpre[:], in0=outpre[:], in1=transb[:], op=ALU.add
                    )

                    # LayerNorm over features (partition dim) via matmul stats
                    sq = workp.tile([HID, NP], F32, tag="sq")
                    nc.scalar.activation(sq[:], outpre[:], AF.Square)
                    hT = actp.tile([HID, NP], F32, tag="hT")
                    for c in range(CH):
                        sl = slice(c * 512, (c + 1) * 512)
                        mu = pmiscp.tile([128, 512], F32, tag="misc")
                        nc.tensor.matmul(
                            out=mu[:], lhsT=ones_m[:], rhs=outpre[:, sl],
                            start=True, stop=True,
                        )
                        ms = pmiscp.tile([128, 512], F32, tag="misc")
                        nc.tensor.matmul(
                            out=ms[:], lhsT=ones_m[:], rhs=sq[:, sl],
                            start=True, stop=True,
                        )
                        mu_sb = workp.tile([128, 512], F32, tag="mu_sb")
                        nc.vector.tensor_copy(out=mu_sb[:], in_=mu[:])
                        var = workp.tile([128, 512], F32, tag="var")
                        nc.vector.tensor_tensor(
                            out=var[:], in0=mu_sb[:], in1=mu_sb[:], op=ALU.mult
                        )
                        nc.vector.scalar_tensor_tensor(
                            out=var[:], in0=ms[:], scalar=1e-5, in1=var[:],
                            op0=ALU.add, op1=ALU.subtract,
                        )
                        lnv = workp.tile([128, 512], F32, tag="lnv")
                        nc.scalar.activation(lnv[:], var[:], AF.Ln)
                        rstd = workp.tile([128, 512], F32, tag="rstd")
                        nc.scalar.activation(rstd[:], lnv[:], AF.Exp, scale=-0.5)
                        xm = workp.tile([128, 512], F32, tag="xm")
                        nc.vector.tensor_tensor(
                            out=xm[:], in0=outpre[:, sl], in1=mu_sb[:], op=ALU.subtract
                        )
                        nc.vector.tensor_tensor(
                            out=xm[:], in0=xm[:], in1=rstd[:], op=ALU.mult
                        )
                        nc.vector.tensor_scalar(
                            out=hT[:, sl], in0=xm[:],
                            scalar1=Wl["gamma", l][:], scalar2=Wl["beta", l][:],
                            op0=ALU.mult, op1=ALU.add,
                        )

                # predict + head
                pT = workp.tile([HID, NP], F32, tag="pT")
                linear(pT, hT, predWT, predb)
                oT = workp.tile([OUT, NP], F32, tag="oT")
                for c in range(CH):
                    sl = slice(c * 512, (c + 1) * 512)
                    ps = pmiscp.tile([OUT, 512], F32, tag="misc")
                    nc.tensor.matmul(
                        out=ps[:], lhsT=headWT[:], rhs=pT[:, sl], start=True, stop=True
                    )
                    nc.scalar.activation(oT[:, sl], ps[:], AF.Identity, bias=headb[:])
                # transpose (OUT, NP) -> (NP, OUT) and store
                for st in range(ST):
                    ps = pmiscp.tile([128, OUT], F32, tag="misc")
                    nc.tensor.matmul(
                        out=ps[:], lhsT=oT[:, st * 128:(st + 1) * 128],
                        rhs=identOUT[:], start=True, stop=True,
                        is_transpose=True,
                    )
                    osb = smallp.tile([128, OUT], F32, tag="osb")
                    nc.vector.tensor_copy(out=osb[:], in_=ps[:])
                    nc.sync.dma_start(
                        out=out_d[b, st * 128:(st + 1) * 128, :], in_=osb[:]
                    )

    _split_sync_waits(nc)
    return nc


# ----------------------------------------------------------------------
# host side
# ----------------------------------------------------------------------

_PROG = None


def _get_program():
    global _PROG
    if _PROG is None:
        _PROG = build_program()
    return _PROG


def _prep_feeds(data, edge_src, edge_dst, params):
    data = np.asarray(data, dtype=np.float32)
    edge_src = np.asarray(edge_src)
    edge_dst = np.asarray(edge_dst)

    def npf(x):
        return np.asarray(x, dtype=np.float32)

    # x: (B, N, T*C) then pad+transpose to (B, 36, NP)
    x = data.transpose(0, 1, 3, 2).reshape(B, N, IN_DIM)
    xT = np.zeros((B, IN_DIM, NP), np.float32)
    xT[:, :, :N] = x.transpose(0, 2, 1)

    # dense log-count masks
    lnC = np.full((R, NP, NP), MASK_NEG, np.float32)
    for r in range(R):
        cnt = np.zeros((NP, NP), np.float32)
        np.add.at(cnt, (edge_src[r], edge_dst[r]), 1.0)
        nz = cnt > 0
        lnC[r][nz] = np.log(cnt[nz])
    import ml_dtypes
    lnC = lnC.reshape(R, ST, 128, NP).astype(ml_dtypes.bfloat16)

    feeds = {
        "lnC": lnC,  # converted to bf16 below
        "adaptWT": np.ascontiguousarray(npf(params["adapt_W"]).T),
        "adaptb": npf(params["adapt_b"]).reshape(HID, 1),
        "predWT": np.ascontiguousarray(npf(params["predict_W"]).T),
        "predb": npf(params["predict_b"]).reshape(HID, 1),
        "headWT": np.ascontiguousarray(npf(params["head_W"]).T),
        "headb": npf(params["head_b"]).reshape(OUT, 1),
        "ieye": np.eye(OUT, dtype=np.float32),
    }

    per_l = {k: [] for k in ("WkT", "WqT", "WvT", "WaT", "bk", "bq", "bv", "ba",
                             "BDatt", "BDmsg", "gamma", "beta", "a1m")}
    for lp in params["layers"]:
        alpha = 1.0 / (1.0 + math.exp(-float(np.asarray(lp["skip"]))))
        per_l["WkT"].append(npf(lp["Wk"]).T)
        per_l["WqT"].append(npf(lp["Wq"]).T)
        per_l["WvT"].append(npf(lp["Wv"]).T)
        per_l["WaT"].append(npf(lp["Wa"]).T * (alpha / R))
        per_l["bk"].append(npf(lp["bk"]).reshape(HID, 1))
        per_l["bq"].append(npf(lp["bq"]).reshape(HID, 1))
        per_l["bv"].append(npf(lp["bv"]).reshape(HID, 1))
        per_l["ba"].append(npf(lp["ba"]).reshape(HID, 1) * alpha)
        per_l["gamma"].append(npf(lp["gamma"]).reshape(HID, 1))
        per_l["beta"].append(npf(lp["beta"]).reshape(HID, 1))
        per_l["a1m"].append(np.full((HID, 1), 1.0 - alpha, np.float32))
        w_att = npf(lp["w_att"])  # (R, H, DK, DK)
        mu = npf(lp["mu"])        # (R, H)
        w_msg = npf(lp["w_msg"])
        bda = np.zeros((R, HID, HID), np.float32)
        bdm = np.zeros((R, HID, HID), np.float32)
        for r in range(R):
            for h in range(H):
                s = slice(h * DK, (h + 1) * DK)
                bda[r][s, s] = (w_att[r, h] * (mu[r, h] / math.sqrt(DK))).T
                bdm[r][s, s] = w_msg[r, h]
        per_l["BDatt"].append(bda)
        per_l["BDmsg"].append(bdm)
    for k, v in per_l.items():
        feeds[k] = np.ascontiguousarray(np.stack(v).astype(np.float32))

    in_maps = []
    for core in range(NCORES):
        m = dict(feeds)
        m["xT"] = np.ascontiguousarray(xT[core * BL:(core + 1) * BL])
        in_maps.append(m)
    return in_maps


def kernel(data, edge_src, edge_dst, params):
    nc = _get_program()
    in_maps = _prep_feeds(data, edge_src, edge_dst, params)
    res = run_bass_kernel_spmd(nc, in_maps, list(range(NCORES)))
    out = np.empty((B, N, OUT), np.float32)
    for core in range(NCORES):
        out[core * BL:(core + 1) * BL] = res.results[core]["out"][:, :N, :]
    return out
